# revision 30
# baseline (speedup 1.0000x reference)
"""Trainium2 Bass kernel for nn_DifferentiableFeatureExtractor.

Strategy (8 NeuronCores, shard T=1048576 along time):
  - per-core extended domain EXT = S + 2048 halo = 133120 = 128 partitions x 1040
  - each partition holds a contiguous chunk plus a 256-bar AP halo
    (tile [128, 1328]); host supplies a 256-bar lead-in so partition 0's halo
    is real data (clamp-padded at the global left edge like the reference)
  - 20 EMAs as *infinite* fp32 recurrences via blocked tensor_tensor_scan:
    per-partition carry from a geometric-weight dot product (STT accum_out)
    + Sh1/M2 PE matmul, then one chained scan2 that starts 8-16 cols before
    the chunk so no boundary fill is needed.  The reference's K-truncation
    is dropped (difference O(c^K) ~ 1e-4 relative, validated numerically).
    EMA8_JX/F1/F2 fuse into one EMA of JXb+6F1+6F2 by linearity when their
    alphas coincide (runtime-guarded).
  - HHV/LLV + RSV front end runs in float16 on per-partition anchored
    H/L/C (anchor = partition's first C), giving the DVE 2x 16-bit mode;
    anchors cancel exactly in (C-ll)/(hh-ll).  fp16 also used for the
    T-ratio/F products, dkd, and cross masks (0/1-exact); JX/EMAJX stay
    fp32 so cross events are stable.
  - rolling std via anchored, column-trimmed doubling window-sums (fp32)
  - BARSLAST/MA_DYNAMIC as segmented scans with affine partition-carry
    chains; the second pass is replaced by a no-event-indicator STT patch;
    exact whenever the previous cross lies within the 2048-bar halo (diag
    output flags violations -> host fallback)
  - engines: DVE does scans/binary ops (the bottleneck, ~99% busy), ACT
    does all unary scaling/conversions, PE the carry matmuls; Pool proved
    ~3-6x slower than DVE for big elementwise tiles and handles only
    setup/DMA.
"""
import math

import numpy as np

import concourse.bacc as bacc
from concourse.bass_types import AP as BassAP
import concourse.mybir as mybir
from concourse import tile as ctile
from concourse.bass_utils import run_bass_kernel_spmd

F32 = mybir.dt.float32
F16 = mybir.dt.float16
Alu = mybir.AluOpType
Act = mybir.ActivationFunctionType

T = 1048576
NCORES = 8
S = T // NCORES            # 131072
P = 128
CH = 1040                  # chunk cols per partition
HP = 256                   # per-partition halo cols
W = HP + CH                # 1328
EXT = P * CH               # 133120
HALO = EXT - S             # 2048
DLEN = HP + EXT            # 133376
C0 = HP                    # chunk start col
NROWS = 30

# static truncation lengths from the reference's constant ALPHAS (used only
# to size the scan1 convergence window)
KS = [72, 72, 72, 286, 286, 286, 559, 89, 54, 47, 40, 30, 130, 30,
      30, 30, 30, 37, 37, 37]


class KB:
    """kernel builder with a simple big-tile freelist"""

    def __init__(self, alphas, anchor):
        self.alphas = [float(a) for a in alphas]
        self.anchor = float(anchor)
        # host-side constant-blob layout (also used by build_const_blob)
        self.VD = {}
        self.gw_off = {}
        off = 0
        for i, a in enumerate(self.alphas):
            K = KS[i]
            lo = 240 if i < 6 else 248
            d = C0 - lo
            V = min(CH, 2 * K + 24) if i < 6 else min(CH, int(1.34 * K) + 12)
            Vd = V - d
            self.VD[i] = Vd
            self.gw_off[i] = off
            off += Vd
        self.oCONSTS = off; off += 20
        self.oSh1 = off; off += P
        self.oIdent = off; off += P
        self.m2_idx = [i for i, a in enumerate(self.alphas)
                       if (1.0 - a) ** CH > 1e-10]
        self.oM2 = {}
        for i in self.m2_idx:
            self.oM2[i] = off; off += P
        self.oM50 = off; off += 1
        for i in range(6, 20):
            self.gw_off[i] = off
            off += self.VD[i]
        self.oGWlate = self.gw_off[6]
        self.oTG = off; off += CH
        self.oTL1 = off; off += CH
        self.CBW = off
        nc = bacc.Bacc(None, target_bir_lowering=False)
        self.nc = nc
        self.CB = nc.dram_tensor("CB", [P * self.CBW], F32, kind="ExternalInput")
        self.DC = nc.dram_tensor("DC", [DLEN], F32, kind="ExternalInput")
        self.DH = nc.dram_tensor("DH", [DLEN], F32, kind="ExternalInput")
        self.DL = nc.dram_tensor("DL", [DLEN], F32, kind="ExternalInput")
        self.OUT = nc.dram_tensor("OUT", [NROWS * EXT], F32, kind="ExternalOutput")
        self.DIAG = nc.dram_tensor("DIAG", [2], F32, kind="ExternalOutput")
        self.free_big = []
        self.n_big = 0
        self.free_big16 = []
        self.n_big16 = 0
        self.free_small = []
        self.n_small = 0
        self.free_row = []
        self.n_row = 0
        self.free_row129 = []
        self.n_row129 = 0

    def build_const_blob(self):
        """host-side [P, CBW] f32 constants matching the layout above."""
        b = np.zeros((P, self.CBW), np.float64)
        for i, a in enumerate(self.alphas):
            Vd = self.VD[i]
            c = 1.0 - a
            o = self.gw_off[i]
            b[:, o : o + Vd] = c ** np.arange(Vd - 1, -1, -1)[None, :]
            b[:, self.oCONSTS + i] = c
        pp = np.arange(P)
        b[:, self.oSh1 : self.oSh1 + P] = (pp[None, :] - pp[:, None] == 1)
        b[:, self.oIdent : self.oIdent + P] = (pp[None, :] - pp[:, None] == 0)
        for i in self.m2_idx:
            cF = (1.0 - self.alphas[i]) ** CH
            o = self.oM2[i]
            b[:, o : o + P] = (pp[None, :] - pp[:, None] == 1) + cF * (
                pp[None, :] - pp[:, None] == 2
            )
        b[:, self.oM50] = -50.0
        b[:, self.oTG : self.oTG + CH] = (
            pp[:, None] * CH + np.arange(CH)[None, :]
        )
        b[:, self.oTL1 : self.oTL1 + CH] = np.arange(1, CH + 1)[None, :]
        return np.ascontiguousarray(b, np.float32).ravel()

    # ---- tile management ----
    def big(self):
        if self.free_big:
            return self.free_big.pop(0)
        t = self.pool.tile([P, W], F32, tag=f"big{self.n_big}")
        self.n_big += 1
        return t

    def rel(self, *ts):
        for t in ts:
            self.free_big.append(t)

    def big16(self):
        if self.free_big16:
            return self.free_big16.pop(0)
        t = self.pool.tile([P, W], F16, tag=f"bigh{self.n_big16}")
        self.n_big16 += 1
        return t

    def rel16(self, *ts):
        for t in ts:
            self.free_big16.append(t)

    def small(self):
        if self.free_small:
            return self.free_small.pop()
        t = self.spool.tile([P, 1], F32, tag=f"small{self.n_small}")
        self.n_small += 1
        return t

    def rels(self, *ts):
        for t in ts:
            self.free_small.append(t)

    def row(self):
        if self.free_row:
            return self.free_row.pop()
        t = self.spool.tile([1, P], F32, tag=f"row{self.n_row}")
        self.n_row += 1
        return t

    def relr(self, *ts):
        for t in ts:
            self.free_row.append(t)

    def row129(self):
        if self.free_row129:
            return self.free_row129.pop()
        t = self.spool.tile([1, P + 1], F32, tag=f"row129_{self.n_row129}")
        self.n_row129 += 1
        return t

    def relr129(self, *ts):
        for t in ts:
            self.free_row129.append(t)

    # ---- IO ----
    def load_series(self, dram, eng="sync", eng2=None):
        nc = self.nc
        t = self.big()
        base = dram[0:DLEN].rearrange("(a b) -> a b", a=1, b=DLEN)
        if eng2 is None:
            src_ap = BassAP(base.tensor, 0, [[CH, P], [1, W]])
            getattr(nc, eng).dma_start(out=t[:, 0:W], in_=src_ap)
        else:
            half = W // 2
            ap1 = BassAP(base.tensor, 0, [[CH, P], [1, half]])
            ap2 = BassAP(base.tensor, half, [[CH, P], [1, W - half]])
            getattr(nc, eng).dma_start(out=t[:, 0:half], in_=ap1)
            getattr(nc, eng2).dma_start(out=t[:, half:W], in_=ap2)
        return t

    def store_row(self, r, t):
        nc = self.nc
        nc.sync.dma_start(
            out=self.OUT[r * EXT : (r + 1) * EXT].rearrange(
                "(p w) -> p w", p=P, w=CH
            ),
            in_=t[:, C0:W],
        )

    # ---- EMA building blocks (infinite recurrence, blocked scan) ----
    def ema_stage1(self, xt, i, lo=248):
        """partition-carry via geometric dot-product (STT accum) + matmul.
        E'[p] = sum_j c^j x[p, W-1-d-j]  (j < Vd), then carry = Sh1/M2 @ E'."""
        nc = self.nc
        d = C0 - lo
        Vd = self.VD[i]
        g = self.GW[i]
        s = self.big()
        ecol = self.small()
        nc.vector.scalar_tensor_tensor(
            out=s[:, W - d - Vd : W - d], in0=g[:, 0:Vd], scalar=1.0,
            in1=xt[:, W - d - Vd : W - d], op0=Alu.mult, op1=Alu.mult,
            accum_out=ecol[:, 0:1],
        )
        mmat = self.M2.get(i, self.Sh1)
        pcar = self.pscol.tile([P, 1], F32, tag="pscol")
        self.mm(pcar[:, 0:1], mmat, ecol[:, 0:1])
        self.rels(ecol)
        return (s, pcar, xt, i, lo)

    def ema_stage2(self, st, scale=1.0):
        """chained scan2 from col lo + in-place ACT scale."""
        nc = self.nc
        s, pcar, xt, i, lo = st
        oc = self.oCONSTS + i
        cbc2 = self.CONSTS_T[:, oc : oc + 1].broadcast_to([P, W - lo])
        nc.vector.tensor_tensor_scan(
            out=s[:, lo:W], data0=cbc2,
            data1=xt[:, lo:W], initial=pcar[:, 0:1], op0=Alu.mult, op1=Alu.add,
        )
        k = self.alphas[i] * scale
        nc.scalar.mul(s[:, lo:W], s[:, lo:W], k)
        return s

    def ema(self, xt, i, scale=1.0, lo=248):
        return self.ema_stage2(self.ema_stage1(xt, i, lo), scale)

    def ema_stage2_raw(self, st):
        """chained scan2 only; caller folds the alpha into a later scale."""
        nc = self.nc
        s, pcar, xt, i, lo = st
        oc = self.oCONSTS + i
        cbc2 = self.CONSTS_T[:, oc : oc + 1].broadcast_to([P, W - lo])
        nc.vector.tensor_tensor_scan(
            out=s[:, lo:W], data0=cbc2,
            data1=xt[:, lo:W], initial=pcar[:, 0:1], op0=Alu.mult, op1=Alu.add,
        )
        return s

    def ema_raw_pair(self, x1, i1, x2, i2, lo=248):
        f1 = self.ema_stage1(x1, i1, lo)
        f2 = self.ema_stage1(x2, i2, lo)
        return self.ema_stage2_raw(f1), self.ema_stage2_raw(f2)

    def scaled(self, s, k, lo=248):
        """materialize k*s into a fresh tile on ACT (off the scan chain)."""
        y = self.big()
        self.nc.scalar.mul(y[:, lo:W], s[:, lo:W], k)
        return y

    def ema_pair(self, x1, i1, x2, i2, scale1=1.0, scale2=1.0, lo=248):
        f1 = self.ema_stage1(x1, i1, lo)
        f2 = self.ema_stage1(x2, i2, lo)
        return self.ema_stage2(f1, scale1), self.ema_stage2(f2, scale2)

    # ---- sliding-window max/min ----
    def winchain(self, xt, jmax, op, alloc=None):
        """doubling chain for sliding max/min: dict 2^j -> (tile, lo)."""
        nc = self.nc
        alloc = alloc or self.big
        chain = {1: (xt, 0)}
        cur, curlo = xt, 0
        for j in range(jmax):
            sh = 1 << j
            dst = alloc()
            nc.vector.tensor_tensor(
                out=dst[:, curlo + sh : W], in0=cur[:, curlo + sh : W],
                in1=cur[:, curlo : W - sh], op=op,
            )
            cur, curlo = dst, curlo + sh
            chain[sh * 2] = (dst, curlo)
        return chain

    def wincombine(self, chain, n, op, alloc=None):
        """window-n result from a doubling chain."""
        nc = self.nc
        alloc = alloc or self.big
        J = 1 << int(math.floor(math.log2(n)))
        r = n - J
        cur, curlo = chain[J]
        out = alloc()
        if r > 0:
            nc.vector.tensor_tensor(
                out=out[:, curlo + r : W], in0=cur[:, curlo + r : W],
                in1=cur[:, curlo : W - r], op=op,
            )
        else:
            nc.vector.tensor_copy(out[:, curlo:W], cur[:, curlo:W])
        return out

    def winsum18(self, xt, need=C0, alloc=None):
        """rolling 18-window sum via doubling, trimmed to cols [need, W).
        W2 needed from need-32 (chain1 read) and each level from its reader."""
        nc = self.nc
        tiles = []
        # start cols per level, derived from the final read pattern
        alloc = alloc or self.big
        s2, s4, s8, s16 = need - 32, need - 24, need - 16, need - 8
        cur = xt
        chain1 = None
        for sh, st in ((1, s2), (2, s4), (4, s8), (8, s16)):
            dst = alloc()
            tiles.append(dst)
            nc.vector.tensor_add(
                dst[:, st:W], cur[:, st - sh : W - sh], cur[:, st:W]
            )
            cur = dst
            if sh == 1:
                chain1 = dst
        out = alloc()
        nc.vector.tensor_add(
            out[:, need:W], cur[:, need:W], chain1[:, need - 16 : W - 16]
        )
        for t in tiles:
            (self.rel if alloc == self.big else self.rel16)(t)
        return out, need

    # ---- full pipeline ----
    def build(self):
        nc = self.nc
        with ctile.TileContext(nc) as tc:
            with tc.tile_pool(name="big", bufs=1) as pool, tc.tile_pool(
                name="small", bufs=1
            ) as spool, tc.tile_pool(name="psc", bufs=4, space="PSUM") as pscol, \
                 tc.tile_pool(name="psr", bufs=2, space="PSUM") as psrow:
                self.pool = pool
                self.spool = spool
                self.pscol = pscol
                self.psrow = psrow
                self.emit()
        nc.finalize()
        return nc

    def mm(self, out, lhsT, rhs):
        self.nc.tensor.matmul(out, lhsT, rhs, start=True, stop=True)

    def emit(self):
        nc = self.nc
        # all big constants (GW weights, Sh1/Ident/M2, scan multipliers,
        # TG/TL1 iotas, m50) are host-precomputed and DMA'd in as one blob,
        # so nothing on the setup path gates the first EMA scans.
        wA = self.oGWlate
        wB = self.CBW - wA
        cbtA = self.spool.tile([P, wA], F32, tag="c_blobA")
        cbtB = self.spool.tile([P, wB], F32, tag="c_blobB")
        base = self.CB[0 : P * self.CBW].rearrange("(p w) -> p w", p=P, w=self.CBW)
        self.GW = {
            i: (cbtA[:, o : o + self.VD[i]] if i < 6 else
                cbtB[:, o - wA : o - wA + self.VD[i]])
            for i, o in self.gw_off.items()
        }
        self.CONSTS_T = cbtA
        self.Sh1 = cbtA[:, self.oSh1 : self.oSh1 + P]
        self.Ident = cbtA[:, self.oIdent : self.oIdent + P]
        self.M2 = {i: cbtA[:, o : o + P] for i, o in self.oM2.items()}
        self.m50 = cbtA[:, self.oM50 : self.oM50 + 1]
        self.TG = cbtB[:, self.oTG - wA : self.oTG - wA + CH]
        self.TL1 = cbtB[:, self.oTL1 - wA : self.oTL1 - wA + CH]
        self.ones11 = self.spool.tile([1, 1], F32, tag="c_ones11")
        nc.gpsimd.memset(self.ones11[:, :], 1.0)
        self.tinyb = self.spool.tile([P, 1], F32, tag="c_tinyb")
        nc.gpsimd.memset(self.tinyb[:, :], 1e-30)

        Ct = self.load_series(self.DC, eng="sync")
        Ht = self.load_series(self.DH, eng="scalar")
        Lt = self.load_series(self.DL, eng="gpsimd")
        nc.scalar.dma_start(out=cbtA[:, 0:wA], in_=base[:, 0:wA])
        nc.gpsimd.dma_start(out=cbtB[:, 0 : self.oTG - wA], in_=base[:, wA : self.oTG])
        nc.gpsimd.dma_start(out=cbtB[:, self.oTG - wA : wB], in_=base[:, self.oTG : self.CBW])

        # fp16 per-partition-anchor conversions first: they only need the
        # loads, and the H/L chains give DVE work while the GW blob lands.
        nb = self.small()
        nc.scalar.mul(nb[:, 0:1], Ct[:, 0:1], -1.0)
        h16 = self.big16()
        nc.scalar.activation(h16[:, 0:W], Ht[:, 0:W], Act.Identity, bias=nb[:, 0:1])
        l16 = self.big16()
        nc.scalar.activation(l16[:, 0:W], Lt[:, 0:W], Act.Identity, bias=nb[:, 0:1])
        c16 = self.big16()
        nc.scalar.activation(c16[:, 0:W], Ct[:, 0:W], Act.Identity, bias=nb[:, 0:1])
        self.rel(Ht, Lt)
        hchain = self.winchain(h16, 7, Alu.max, alloc=self.big16)
        lchain = self.winchain(l16, 7, Alu.min, alloc=self.big16)

        # --- rolling std of C (window 18), fp16 with per-partition anchor ---
        # dev = C - C[partition col 0]; var is shift-invariant, and the
        # TEU3/TED rows tolerate ~2 abs error, so fp16 sums are safe here.
        nb18 = self.small()
        nc.scalar.mul(nb18[:, 0:1], Ct[:, 0:1], -1.0 / 18.0)
        nbr18 = self.small()
        nc.scalar.mul(nbr18[:, 0:1], Ct[:, 0:1], -1.0 / math.sqrt(18.0))
        dev_s = self.big16()
        nc.scalar.activation(
            dev_s[:, 0:W], Ct[:, 0:W], Act.Identity, bias=nb18[:, 0:1],
            scale=1.0 / 18.0,
        )
        dev2 = self.big16()
        nc.scalar.activation(
            dev2[:, 0:W], Ct[:, 0:W], Act.Square, bias=nbr18[:, 0:1],
            scale=1.0 / math.sqrt(18.0),
        )
        m18, lo1 = self.winsum18(dev_s, alloc=self.big16)
        q18, lo2 = self.winsum18(dev2, alloc=self.big16)
        self.rel16(dev_s, dev2)
        mm2 = self.big16()
        nc.scalar.square(mm2[:, lo1:W], m18[:, lo1:W])
        var = self.big16()
        nc.vector.tensor_sub(var[:, lo1:W], q18[:, lo1:W], mm2[:, lo1:W])
        nc.vector.tensor_scalar_max(var[:, lo1:W], var[:, lo1:W], 0.0)
        DIS = self.big()
        nc.scalar.activation(DIS[:, lo1:W], var[:, lo1:W], Act.Sqrt)
        self.rel16(m18, q18, mm2, var)
        self.rels(nb18, nbr18)

        # --- TEMA3 + TEMAP2 chains (raw scans; alphas folded at the end) ---
        aa = self.alphas
        s1, t1 = self.ema_raw_pair(Ct, 0, Ct, 3, lo=240)
        s2, t2 = self.ema_raw_pair(s1, 1, t1, 4, lo=240)
        s3, t3 = self.ema_raw_pair(s2, 2, t2, 5, lo=240)

        EMA1 = self.scaled(s1, aa[0], lo=240)
        EMA2 = self.scaled(s2, aa[0] * aa[1], lo=240)
        EMA3 = self.scaled(s3, aa[0] * aa[1] * aa[2], lo=240)
        self.rel(s1, s2, s3)
        E21 = self.scaled(t1, aa[3], lo=240)
        E221 = self.scaled(t2, aa[3] * aa[4], lo=240)
        E231 = self.scaled(t3, aa[3] * aa[4] * aa[5], lo=240)
        self.rel(t1, t2, t3)
        TEMA3 = self.big()
        d = self.big()
        nc.vector.tensor_sub(d[:, 240:W], EMA1[:, 240:W], EMA2[:, 240:W])
        nc.vector.scalar_tensor_tensor(
            out=TEMA3[:, 240:W], in0=d[:, 240:W], scalar=3.0, in1=EMA3[:, 240:W],
            op0=Alu.mult, op1=Alu.add,
        )
        self.rel(EMA1, EMA2, EMA3, d)
        self.store_row(4, TEMA3)
        TEMAP2 = self.big()
        d = self.big()
        nc.vector.tensor_sub(d[:, 240:W], E21[:, 240:W], E221[:, 240:W])
        nc.vector.scalar_tensor_tensor(
            out=TEMAP2[:, 240:W], in0=d[:, 240:W], scalar=3.0, in1=E231[:, 240:W],
            op0=Alu.mult, op1=Alu.add,
        )
        self.rel(E21, E221, E231, d)

        # --- KDJ blocks (fp16, per-partition anchored) ---

        def rsvf(nw):
            """rsv in [0,100] via (C-ll)/(hh-ll)*100, fp16 anchored front end
            (the *100 is folded into the K-EMA scale)."""
            hh = self.wincombine(hchain, nw, Alu.max, alloc=self.big16)
            ll = self.wincombine(lchain, nw, Alu.min, alloc=self.big16)
            hl = self.big16()
            nc.vector.tensor_sub(hl[:, 248:W], hh[:, 248:W], ll[:, 248:W])
            hlf = self.big()
            nc.scalar.copy(hlf[:, 248:W], hl[:, 248:W])
            rcp = self.big()
            nc.vector.reciprocal_approx_fast(out=rcp[:, 248:W], in_=hlf[:, 248:W])
            rcp16 = self.big16()
            nc.scalar.copy(rcp16[:, 248:W], rcp[:, 248:W])
            num = self.big16()
            nc.vector.tensor_sub(num[:, 248:W], c16[:, 248:W], ll[:, 248:W])
            r0 = self.big16()
            nc.vector.tensor_mul(r0[:, 248:W], num[:, 248:W], rcp16[:, 248:W])
            self.rel(hlf, rcp)
            self.rel16(hh, ll, num, hl, rcp16)
            return r0

        def jof(Kv, Dv, rows):
            """J = 3K - 2D = 2*(K-D) + K; store K/D/J rows."""
            K16 = self.big16()
            nc.scalar.copy(K16[:, 248:W], Kv[:, 248:W])
            D16 = self.big16()
            nc.scalar.copy(D16[:, 248:W], Dv[:, 248:W])
            dkd = self.big16()
            nc.vector.tensor_sub(dkd[:, 248:W], K16[:, 248:W], D16[:, 248:W])
            self.rel16(K16, D16)
            Jv = self.big()
            nc.vector.scalar_tensor_tensor(
                out=Jv[:, 248:W], in0=dkd[:, 248:W], scalar=2.0, in1=Kv[:, 248:W],
                op0=Alu.mult, op1=Alu.add,
            )
            self.rel16(dkd)
            for ridx, tt_ in zip(rows, (Kv, Dv, Jv)):
                if ridx is not None:
                    self.store_row(ridx, tt_)
            return Jv

        a = self.alphas
        rsv1 = rsvf(204)
        rsv2 = rsvf(18)
        sK1, sK2 = self.ema_raw_pair(rsv1, 6, rsv2, 8)
        K1 = self.scaled(sK1, 100.0 * a[6])
        K2 = self.scaled(sK2, 100.0 * a[8])
        self.rel16(rsv1, rsv2)
        rsv3 = rsvf(9)
        rsvn = rsvf(36)
        sK3, sKN3 = self.ema_raw_pair(rsv3, 10, rsvn, 12)
        K3 = self.scaled(sK3, 100.0 * a[10])
        KN3 = self.scaled(sKN3, 100.0 * a[12])
        self.rel16(rsv3, rsvn)
        f1 = self.ema_stage1(sK1, 7)
        f2 = self.ema_stage1(sK2, 9)
        f3 = self.ema_stage1(sK3, 11)
        f4 = self.ema_stage1(sKN3, 13)
        D1 = self.ema_stage2(f1, 100.0 * a[6])
        D2 = self.ema_stage2(f2, 100.0 * a[8])
        D3 = self.ema_stage2(f3, 100.0 * a[10])
        DN3m2 = self.ema_stage2(f4, -2.0 * 100.0 * a[12])
        self.rel(sK1, sK2, sK3, sKN3)
        J1 = jof(K1, D1, (9, 10, 11))
        J2 = jof(K2, D2, (12, 13, 14))
        self.rel(K1, D1, K2, D2)
        J3 = jof(K3, D3, (15, 16, 17))
        JN3 = self.big()
        nc.vector.scalar_tensor_tensor(
            out=JN3[:, C0:W], in0=KN3[:, C0:W], scalar=3.0, in1=DN3m2[:, C0:W],
            op0=Alu.mult, op1=Alu.add,
        )
        self.store_row(18, JN3)
        self.rel(K3, D3, KN3, DN3m2, JN3)
        for ch_ in (hchain, lchain):
            for kk, (tt_, _) in ch_.items():
                if kk > 1:
                    self.rel16(tt_)
        self.rel16(h16, l16, c16)

        TEU3 = self.big()
        nc.vector.tensor_add(TEU3[:, C0:W], TEMA3[:, C0:W], DIS[:, C0:W])
        TED = self.big()
        nc.vector.tensor_sub(TED[:, C0:W], TEMA3[:, C0:W], DIS[:, C0:W])
        self.store_row(3, TEU3)
        self.store_row(5, TED)
        self.rel(DIS, TEU3, TED)

        # --- T ratios ---
        def recip_abs(xt, lo):
            ab = self.big()
            nc.scalar.activation(ab[:, lo:W], xt[:, lo:W], Act.Abs)
            rr = self.big()
            nc.vector.reciprocal_approx_fast(out=rr[:, lo:W], in_=ab[:, lo:W])
            self.rel(ab)
            return rr

        def tdiff(xt, lag, row_idx, rr16, lo=248):
            # rr16 = fp16(1/|xt|) (unshifted); read it lag-shifted
            dt_ = self.big16()
            nc.vector.tensor_sub(
                dt_[:, lo:W], xt[:, lo:W], xt[:, lo - lag : W - lag]
            )
            ts_ = self.big16()
            nc.vector.tensor_mul(ts_[:, lo:W], dt_[:, lo:W], rr16[:, lo - lag : W - lag])
            self.rel16(dt_)
            if row_idx is not None:
                tsf = self.big()
                nc.scalar.copy(tsf[:, C0:W], ts_[:, C0:W])
                self.store_row(row_idx, tsf)
                self.rel(tsf)
            return ts_

        rrA = recip_abs(TEMA3, 242)
        rrP = recip_abs(TEMAP2, 242)
        rrA16 = self.big16()
        nc.scalar.copy(rrA16[:, 242:W], rrA[:, 242:W])
        rrP16 = self.big16()
        nc.scalar.copy(rrP16[:, 242:W], rrP[:, 242:W])
        self.rel(rrA, rrP)
        T3s = tdiff(TEMA3, 6, 8, rrA16)
        T1s = tdiff(TEMA3, 1, 6, rrA16)
        T2s = tdiff(TEMAP2, 6, 7, rrP16)
        self.rel(TEMAP2, TEMA3)
        self.rel16(rrA16, rrP16)

        # --- JX family (J/T in fp16; JXb kept f32 for cross stability) ---
        J1_16 = self.big16()
        nc.scalar.copy(J1_16[:, 248:W], J1[:, 248:W])
        J2_16 = self.big16()
        nc.scalar.copy(J2_16[:, 248:W], J2[:, 248:W])
        J3_16 = self.big16()
        nc.scalar.copy(J3_16[:, 248:W], J3[:, 248:W])
        JXb = self.big()
        u = self.big16()
        nc.vector.tensor_mul(u[:, 248:W], J3_16[:, 248:W], T1s[:, 248:W])
        v = self.big()
        nc.vector.tensor_add(v[:, 248:W], J1[:, 248:W], J2[:, 248:W])
        nc.vector.tensor_add(JXb[:, 248:W], u[:, 248:W], v[:, 248:W])
        self.rel(v, J3, J1, J2)
        self.rel16(u, J3_16, T1s)
        F1 = self.big16()
        nc.vector.tensor_mul(F1[:, 248:W], J2_16[:, 248:W], T3s[:, 248:W])
        self.rel16(J2_16, T3s)
        F2 = self.big16()
        nc.vector.tensor_mul(F2[:, 248:W], J1_16[:, 248:W], T2s[:, 248:W])
        self.rel16(J1_16, T2s)
        self.store_row(19, JXb)
        F1f = self.big()
        nc.scalar.copy(F1f[:, C0:W], F1[:, C0:W])
        self.store_row(20, F1f)
        F2f = self.big()
        nc.scalar.copy(F2f[:, C0:W], F2[:, C0:W])
        self.store_row(21, F2f)
        self.rel(F1f, F2f)

        g1 = self.ema_stage1(JXb, 14)
        g2 = self.ema_stage1(F1, 15)
        g3 = self.ema_stage1(F2, 16)
        EMA_JX = self.ema_stage2(g1)
        EMA_F1 = self.ema_stage2(g2)
        EMA_F2 = self.ema_stage2(g3)
        self.store_row(22, EMA_JX)
        self.store_row(23, EMA_F1)
        self.store_row(24, EMA_F2)

        def jx_combine(base, f1, f2, row_idx, lo=254, keep_z=False):
            w_ = self.big()
            nc.vector.tensor_add(w_[:, lo:W], f1[:, lo:W], f2[:, lo:W])
            z = self.big()
            nc.vector.scalar_tensor_tensor(
                out=z[:, lo:W], in0=w_[:, lo:W], scalar=6.0, in1=base[:, lo:W],
                op0=Alu.mult, op1=Alu.add,
            )
            out = self.big()
            nc.scalar.activation(out[:, lo:W], z[:, lo:W], Act.Identity, bias=self.m50)
            self.rel(w_)
            if not keep_z:
                self.rel(z)
                z = None
            self.store_row(row_idx, out)
            return out, z

        # z_JX = JXb + 6*(F1+F2); EMAJX8 = a17*scan(z_JX) - 50 when the three
        # EMA8 alphas coincide (they do for the reference w_alphas); otherwise
        # fall back to three separate EMAs.
        a17, a18, a19 = self.alphas[17], self.alphas[18], self.alphas[19]
        fuse8 = abs(a17 - a18) < 1e-9 and abs(a17 - a19) < 1e-9
        JX, zJX = jx_combine(JXb, F1, F2, 27, lo=248, keep_z=True)
        if fuse8:
            st8 = self.ema_stage1(zJX, 17)
            oc8 = self.oCONSTS + 17
            cbc8 = self.CONSTS_T[:, oc8 : oc8 + 1].broadcast_to([P, W - 248])
            s8, pcar8 = st8[0], st8[1]
            nc.vector.tensor_tensor_scan(
                out=s8[:, 248:W], data0=cbc8, data1=zJX[:, 248:W],
                initial=pcar8[:, 0:1], op0=Alu.mult, op1=Alu.add,
            )
            EMAJX8 = self.big()
            nc.scalar.activation(
                EMAJX8[:, 248:W], s8[:, 248:W], Act.Identity,
                bias=self.m50, scale=a17,
            )
            self.rel(s8)
            self.store_row(29, EMAJX8)
        else:
            EMA8_JX = self.ema(JXb, 17)
            EMA8_F1 = self.ema(F1, 18)
            EMA8_F2 = self.ema(F2, 19)
            EMAJX8, _ = jx_combine(EMA8_JX, EMA8_F1, EMA8_F2, 29, lo=254)
            self.rel(EMA8_JX, EMA8_F1, EMA8_F2)
        self.rel(zJX)
        EMAJX, _ = jx_combine(EMA_JX, EMA_F1, EMA_F2, 28, lo=254)
        self.rel(JXb, EMA_JX, EMA_F1, EMA_F2)
        self.rel16(F1, F2)
        self.rel(EMAJX8)

        # --- crosses + segmented MA scans ---
        def macond(updown):
            """m_ = 1 - cond where cond = cross event mask.
            cross_up[t] = (JX>EMAJX)[t] AND NOT (JX>EMAJX)[t-1]  (is_le == !is_gt)
            cross_dn[t] = (JX<EMAJX)[t] AND NOT (JX<EMAJX)[t-1]"""
            g = self.big16()
            op = Alu.is_gt if updown == "up" else Alu.is_lt
            nc.vector.tensor_tensor(
                out=g[:, 254:W], in0=JX[:, 254:W], in1=EMAJX[:, 254:W], op=op,
            )
            w1 = self.big16()
            nc.vector.tensor_mul(w1[:, 255:W], g[:, 255:W], g[:, 254 : W - 1])
            m_ = self.big16()
            nc.vector.scalar_tensor_tensor(
                out=m_[:, 255:W], in0=w1[:, 255:W], scalar=1.0, in1=g[:, 255:W],
                op0=Alu.add, op1=Alu.subtract,
            )
            self.rel16(g, w1)
            return m_

        def ma_phase1(updown):
            m_ = macond(updown)
            dmask = self.big()
            nc.vector.tensor_mul(dmask[:, C0:W], Ct[:, C0:W], m_[:, C0:W])
            cnt_s = self.big()
            nc.vector.tensor_tensor_scan(
                out=cnt_s[:, C0:W], data0=m_[:, C0:W], data1=m_[:, C0:W],
                initial=0.0, op0=Alu.mult, op1=Alu.add,
            )
            acol = self.small()
            nc.vector.tensor_single_scalar(
                out=acol[:, 0:1], in_=cnt_s[:, W - 1 : W], scalar=float(CH),
                op=Alu.is_ge,
            )
            par = self.psrow.tile([1, P], F32, tag="psrow")
            self.mm(par[0:1, 0:P], acol[:, 0:1], self.Ident)
            arow = self.row()
            nc.vector.tensor_copy(arow[0:1, 0:P], par[0:1, 0:P])
            self.rels(acol)
            Ssum = self.big()
            nc.vector.tensor_tensor_scan(
                out=Ssum[:, C0:W], data0=m_[:, C0:W], data1=dmask[:, C0:W],
                initial=0.0, op0=Alu.mult, op1=Alu.add,
            )

            def carry_of(scan1_tile):
                rowt = self.row129()
                nc.gpsimd.memset(rowt[0:1, 0:1], 0.0)
                pbr = self.psrow.tile([1, P], F32, tag="psrow")
                self.mm(pbr[0:1, 0:P], scan1_tile[:, W - 1 : W], self.Ident)
                nc.vector.tensor_tensor_scan(
                    out=rowt[0:1, 1 : P + 1], data0=arow[0:1, 0:P],
                    data1=pbr[0:1, 0:P], initial=0.0, op0=Alu.mult, op1=Alu.add,
                )
                pcc = self.pscol.tile([P, 1], F32, tag="pscol")
                self.mm(pcc[:, 0:1], rowt[0:1, 0:P], self.ones11[0:1, 0:1])
                car = self.small()
                nc.vector.tensor_copy(car[:, 0:1], pcc[:, 0:1])
                self.relr129(rowt)
                return car

            carc = carry_of(cnt_s)
            cars = carry_of(Ssum)
            noev = self.big()
            nc.vector.tensor_tensor(
                out=noev[:, C0:W], in0=cnt_s[:, C0:W], in1=self.TL1,
                op=Alu.is_equal,
            )
            self.relr(arow)
            self.rel16(m_)
            self.rel(dmask)
            return dict(cnt_s=cnt_s, Ssum=Ssum, carc=carc, cars=cars, noev=noev)

        def ma_phase2(st, row_idx, diag_idx):
            cnt_s, Ssum, noev = st["cnt_s"], st["Ssum"], st["noev"]
            nc.vector.scalar_tensor_tensor(
                out=cnt_s[:, C0:W], in0=noev[:, C0:W], scalar=st["carc"][:, 0:1],
                in1=cnt_s[:, C0:W], op0=Alu.mult, op1=Alu.add,
            )
            nc.vector.scalar_tensor_tensor(
                out=Ssum[:, C0:W], in0=noev[:, C0:W], scalar=st["cars"][:, 0:1],
                in1=Ssum[:, C0:W], op0=Alu.mult, op1=Alu.add,
            )
            self.rels(st["carc"], st["cars"])
            seen = self.big()
            dcol = self.small()
            nc.vector.scalar_tensor_tensor(
                out=seen[:, C0:W], in0=cnt_s[:, C0:W], scalar=1.0,
                in1=self.TG, op0=Alu.mult, op1=Alu.is_le,
                accum_out=dcol[:, 0:1],
            )
            self.rel(noev)
            # diag early so its DMA/reduce overlaps the ma arithmetic
            drow = self.row()
            nc.sync.dma_start(out=drow[0:1, 0 : P - 1], in_=dcol[1:P, 0:1])
            done = self.spool.tile([1, 1], F32, tag=f"diag{diag_idx}", name=f"diag{diag_idx}")
            nc.vector.tensor_reduce(
                out=done[0:1, 0:1], in_=drow[0:1, 0 : P - 1],
                axis=mybir.AxisListType.X, op=Alu.min,
            )
            self.relr(drow)
            nc.sync.dma_start(
                out=self.DIAG[diag_idx : diag_idx + 1].rearrange(
                    "(a b) -> a b", a=1, b=1
                ),
                in_=done[0:1, 0:1],
            )
            self.rels(dcol)
            rc = self.big()
            nc.vector.tensor_scalar_max(rc[:, C0:W], cnt_s[:, C0:W], 1.0)
            rcp = self.big()
            nc.vector.reciprocal_approx_fast(out=rcp[:, C0:W], in_=rc[:, C0:W])
            ma0 = self.big()
            nc.vector.tensor_mul(ma0[:, C0:W], Ssum[:, C0:W], rcp[:, C0:W])
            ma = self.big()
            nc.vector.tensor_mul(ma[:, C0:W], ma0[:, C0:W], seen[:, C0:W])
            self.rel(rc, rcp, ma0, cnt_s, Ssum)
            self.store_row(row_idx, ma)
            self.rel(seen, ma)

        stA = ma_phase1("dn")
        stB = ma_phase1("up")
        ma_phase2(stA, 25, 1)
        ma_phase2(stB, 26, 0)
        self.rel(Ct, JX, EMAJX)


_CACHE = {}


def _build(alphas, anchor):
    key = (tuple(round(float(a), 12) for a in alphas), round(float(anchor), 6))
    if key not in _CACHE:
        kb = KB(alphas, anchor)
        _CACHE[key] = (kb.build(), kb.build_const_blob())
    return _CACHE[key]


def _shard(x):
    """per-core input arrays [DLEN], clamp-padded on the global left."""
    outs = []
    for mcore in range(NCORES):
        lo = (mcore + 1) * S - DLEN
        if lo < 0:
            d = np.concatenate(
                [np.full(-lo, x[0], np.float32), x[0 : (mcore + 1) * S]]
            )
        else:
            d = x[lo : (mcore + 1) * S]
        outs.append(np.ascontiguousarray(d, np.float32))
    return outs


def _host_ma(C, JX, EJ):
    """exact host fallback for ma rows (numpy, global)."""
    f32 = np.float32
    T_ = len(C)
    lag = lambda x: np.concatenate([x[:1], x[:-1]])
    JXp, EJp = lag(JX), lag(EJ)
    res = {}
    cs = np.concatenate([[0.0], np.cumsum(C.astype(np.float64))])
    t_idx = np.arange(T_)
    for key, cond in (
        ("dn", (JX < EJ) & (JXp >= EJp)),
        ("up", (JX > EJ) & (JXp <= EJp)),
    ):
        last = np.maximum.accumulate(np.where(cond, t_idx, -1))
        csl = cs[np.maximum(last, 0) + 1]
        s = cs[t_idx + 1] - csl
        n = t_idx - last
        res[key] = np.where(
            (last >= 0) & (n > 0), s / np.maximum(n, 1), 0.0
        ).astype(f32)
    return res["dn"], res["up"]


def run_cores(inputs, trace=False):
    """compile (cached) + run on 8 cores; returns BassKernelResults."""
    C = np.ascontiguousarray(inputs["C"], np.float32)
    H = np.ascontiguousarray(inputs["H"], np.float32)
    L = np.ascontiguousarray(inputs["L"], np.float32)
    w = np.asarray(inputs["w_alphas"], np.float32)
    alphas = [float(1.0 / (1.0 + math.exp(-float(x)))) for x in w]
    nc, cb = _build(alphas, float(C[0]))
    dc, dh, dl = _shard(C), _shard(H), _shard(L)
    in_maps = [
        {"DC": dc[m], "DH": dh[m], "DL": dl[m], "CB": cb} for m in range(NCORES)
    ]
    res = run_bass_kernel_spmd(
        nc, in_maps, core_ids=list(range(NCORES)), trace=trace
    )
    return res


def kernel(C, H, L, w_alphas):
    inputs = {"C": C, "H": H, "L": L, "w_alphas": w_alphas}
    res = run_cores(inputs)
    outs = [res.results[m]["OUT"].reshape(NROWS, EXT)[:, HALO:] for m in range(NCORES)]
    full = np.concatenate(outs, axis=1)
    full[0] = np.asarray(C, np.float32)
    full[1] = np.asarray(H, np.float32)
    full[2] = np.asarray(L, np.float32)

    # host patch: reference's partial-window std for the first 17 bars
    Cg = np.asarray(C, np.float64)[:17]
    for t in range(17):
        wdw = Cg[: t + 1]
        dis = math.sqrt(max(np.mean(wdw * wdw) - np.mean(wdw) ** 2, 0.0))
        full[3, t] = np.float32(full[4, t] + dis)
        full[5, t] = np.float32(full[4, t] - dis)

    # diag check: cross gap exceeded the halo on some core -> exact host fix
    need_fix = False
    for mcore in range(1, NCORES):
        dg = res.results[mcore]["DIAG"]
        if dg.min() < CH - 0.5:
            need_fix = True
    if need_fix:
        ma_dn, ma_up = _host_ma(
            np.asarray(C, np.float32), full[27], full[28]
        )
        full[25] = ma_dn
        full[26] = ma_up
    return full.astype(np.float32)


# revision 31
# speedup vs baseline: 1.1889x; 1.1889x over previous
"""Trainium2 Bass kernel for nn_DifferentiableFeatureExtractor.

Strategy (8 NeuronCores, shard T=1048576 along time):
  - per-core extended domain EXT = S + 2048 halo = 133120 = 128 partitions x 1040
  - each partition holds a contiguous chunk plus a 256-bar AP halo
    (tile [128, 1328]); host supplies a 256-bar lead-in so partition 0's halo
    is real data (clamp-padded at the global left edge like the reference)
  - 20 EMAs as *infinite* fp32 recurrences via blocked tensor_tensor_scan:
    per-partition carry from a geometric-weight dot product (STT accum_out)
    + Sh1/M2 PE matmul, then one chained scan2 that starts 8-16 cols before
    the chunk so no boundary fill is needed.  The reference's K-truncation
    is dropped (difference O(c^K) ~ 1e-4 relative, validated numerically).
    EMA8_JX/F1/F2 fuse into one EMA of JXb+6F1+6F2 by linearity when their
    alphas coincide (runtime-guarded).
  - HHV/LLV + RSV front end runs in float16 on per-partition anchored
    H/L/C (anchor = partition's first C), giving the DVE 2x 16-bit mode;
    anchors cancel exactly in (C-ll)/(hh-ll).  fp16 also used for the
    T-ratio/F products, dkd, and cross masks (0/1-exact); JX/EMAJX stay
    fp32 so cross events are stable.
  - rolling std via anchored, column-trimmed doubling window-sums (fp32)
  - BARSLAST/MA_DYNAMIC as segmented scans with affine partition-carry
    chains; the second pass is replaced by a no-event-indicator STT patch;
    exact whenever the previous cross lies within the 2048-bar halo (diag
    output flags violations -> host fallback)
  - engines: DVE does scans/binary ops (the bottleneck, ~99% busy), ACT
    does all unary scaling/conversions, PE the carry matmuls; Pool proved
    ~3-6x slower than DVE for big elementwise tiles and handles only
    setup/DMA.
"""
import math

import numpy as np

import concourse.bacc as bacc
from concourse.bass_types import AP as BassAP
import concourse.mybir as mybir
from concourse import tile as ctile
from concourse.bass_utils import run_bass_kernel_spmd

F32 = mybir.dt.float32
F16 = mybir.dt.float16
Alu = mybir.AluOpType
Act = mybir.ActivationFunctionType

T = 1048576
NCORES = 8
S = T // NCORES            # 131072
P = 128
CH = 1040                  # chunk cols per partition
HP = 256                   # per-partition halo cols
W = HP + CH                # 1328
EXT = P * CH               # 133120
HALO = EXT - S             # 2048
DLEN = HP + EXT            # 133376
C0 = HP                    # chunk start col
NROWS = 30

# static truncation lengths from the reference's constant ALPHAS (used only
# to size the scan1 convergence window)
KS = [72, 72, 72, 286, 286, 286, 559, 89, 54, 47, 40, 30, 130, 30,
      30, 30, 30, 37, 37, 37]


class KB:
    """kernel builder with a simple big-tile freelist"""

    def __init__(self, alphas, anchor):
        self.alphas = [float(a) for a in alphas]
        self.anchor = float(anchor)
        # host-side constant-blob layout (also used by build_const_blob)
        self.VD = {}
        self.gw_off = {}
        off = 0
        for i, a in enumerate(self.alphas):
            K = KS[i]
            lo = 240 if i < 6 else 248
            d = C0 - lo
            V = min(CH, 2 * K + 24) if i < 6 else min(CH, int(1.34 * K) + 12)
            Vd = V - d
            self.VD[i] = Vd
            self.gw_off[i] = off
            off += Vd
        self.oCONSTS = off; off += 20
        self.oSh1 = off; off += P
        self.oIdent = off; off += P
        self.m2_idx = [i for i, a in enumerate(self.alphas)
                       if (1.0 - a) ** CH > 1e-10]
        self.oM2 = {}
        for i in self.m2_idx:
            self.oM2[i] = off; off += P
        self.oM50 = off; off += 1
        for i in range(6, 20):
            self.gw_off[i] = off
            off += self.VD[i]
        self.oGWlate = self.gw_off[6]
        self.oTG = off; off += CH
        self.oTL1 = off; off += CH
        self.CBW = off
        nc = bacc.Bacc(None, target_bir_lowering=False)
        self.nc = nc
        self.CB = nc.dram_tensor("CB", [P * self.CBW], F32, kind="ExternalInput")
        self.DC = nc.dram_tensor("DC", [DLEN], F32, kind="ExternalInput")
        self.DH = nc.dram_tensor("DH", [DLEN], F32, kind="ExternalInput")
        self.DL = nc.dram_tensor("DL", [DLEN], F32, kind="ExternalInput")
        self.OUT = nc.dram_tensor("OUT", [NROWS * EXT], F32, kind="ExternalOutput")
        self.DIAG = nc.dram_tensor("DIAG", [2], F32, kind="ExternalOutput")
        self.free_big = []
        self.n_big = 0
        self.free_big16 = []
        self.n_big16 = 0
        self.free_small = []
        self.n_small = 0
        self.free_row = []
        self.n_row = 0
        self.free_row129 = []
        self.n_row129 = 0

    def build_const_blob(self):
        """host-side [P, CBW] f32 constants matching the layout above."""
        b = np.zeros((P, self.CBW), np.float64)
        for i, a in enumerate(self.alphas):
            Vd = self.VD[i]
            c = 1.0 - a
            o = self.gw_off[i]
            b[:, o : o + Vd] = c ** np.arange(Vd - 1, -1, -1)[None, :]
            b[:, self.oCONSTS + i] = c
        pp = np.arange(P)
        b[:, self.oSh1 : self.oSh1 + P] = (pp[None, :] - pp[:, None] == 1)
        b[:, self.oIdent : self.oIdent + P] = (pp[None, :] - pp[:, None] == 0)
        for i in self.m2_idx:
            cF = (1.0 - self.alphas[i]) ** CH
            o = self.oM2[i]
            b[:, o : o + P] = (pp[None, :] - pp[:, None] == 1) + cF * (
                pp[None, :] - pp[:, None] == 2
            )
        b[:, self.oM50] = -50.0
        b[:, self.oTG : self.oTG + CH] = (
            pp[:, None] * CH + np.arange(CH)[None, :]
        )
        b[:, self.oTL1 : self.oTL1 + CH] = np.arange(1, CH + 1)[None, :]
        return np.ascontiguousarray(b, np.float32).ravel()

    # ---- tile management ----
    def big(self):
        if self.free_big:
            return self.free_big.pop(0)
        t = self.pool.tile([P, W], F32, tag=f"big{self.n_big}")
        self.n_big += 1
        return t

    def rel(self, *ts):
        for t in ts:
            self.free_big.append(t)

    def big16(self):
        if self.free_big16:
            return self.free_big16.pop(0)
        t = self.pool.tile([P, W], F16, tag=f"bigh{self.n_big16}")
        self.n_big16 += 1
        return t

    def rel16(self, *ts):
        for t in ts:
            self.free_big16.append(t)

    def small(self):
        if self.free_small:
            return self.free_small.pop()
        t = self.spool.tile([P, 1], F32, tag=f"small{self.n_small}")
        self.n_small += 1
        return t

    def rels(self, *ts):
        for t in ts:
            self.free_small.append(t)

    def row(self):
        if self.free_row:
            return self.free_row.pop()
        t = self.spool.tile([1, P], F32, tag=f"row{self.n_row}")
        self.n_row += 1
        return t

    def relr(self, *ts):
        for t in ts:
            self.free_row.append(t)

    def row129(self):
        if self.free_row129:
            return self.free_row129.pop()
        t = self.spool.tile([1, P + 1], F32, tag=f"row129_{self.n_row129}")
        self.n_row129 += 1
        return t

    def relr129(self, *ts):
        for t in ts:
            self.free_row129.append(t)

    # ---- IO ----
    def load_series(self, dram, eng="sync", eng2=None):
        nc = self.nc
        t = self.big()
        base = dram[0:DLEN].rearrange("(a b) -> a b", a=1, b=DLEN)
        if eng2 is None:
            src_ap = BassAP(base.tensor, 0, [[CH, P], [1, W]])
            getattr(nc, eng).dma_start(out=t[:, 0:W], in_=src_ap)
        else:
            half = W // 2
            ap1 = BassAP(base.tensor, 0, [[CH, P], [1, half]])
            ap2 = BassAP(base.tensor, half, [[CH, P], [1, W - half]])
            getattr(nc, eng).dma_start(out=t[:, 0:half], in_=ap1)
            getattr(nc, eng2).dma_start(out=t[:, half:W], in_=ap2)
        return t

    def store_row(self, r, t):
        nc = self.nc
        nc.sync.dma_start(
            out=self.OUT[r * EXT : (r + 1) * EXT].rearrange(
                "(p w) -> p w", p=P, w=CH
            ),
            in_=t[:, C0:W],
        )

    # ---- EMA building blocks (infinite recurrence, blocked scan) ----
    def ema_stage1(self, xt, i, lo=248):
        """partition-carry via geometric dot-product (STT accum) + matmul.
        E'[p] = sum_j c^j x[p, W-1-d-j]  (j < Vd), then carry = Sh1/M2 @ E'."""
        nc = self.nc
        d = C0 - lo
        Vd = self.VD[i]
        g = self.GW[i]
        s = self.big()
        ecol = self.small()
        nc.vector.scalar_tensor_tensor(
            out=s[:, W - d - Vd : W - d], in0=g[:, 0:Vd], scalar=1.0,
            in1=xt[:, W - d - Vd : W - d], op0=Alu.mult, op1=Alu.mult,
            accum_out=ecol[:, 0:1],
        )
        mmat = self.M2.get(i, self.Sh1)
        pcar = self.pscol.tile([P, 1], F32, tag="pscol")
        self.mm(pcar[:, 0:1], mmat, ecol[:, 0:1])
        self.rels(ecol)
        return (s, pcar, xt, i, lo)

    def ema_stage2(self, st, scale=1.0):
        """chained scan2 from col lo + in-place ACT scale."""
        nc = self.nc
        s, pcar, xt, i, lo = st
        oc = self.oCONSTS + i
        cbc2 = self.CONSTS_T[:, oc : oc + 1].broadcast_to([P, W - lo])
        nc.vector.tensor_tensor_scan(
            out=s[:, lo:W], data0=cbc2,
            data1=xt[:, lo:W], initial=pcar[:, 0:1], op0=Alu.mult, op1=Alu.add,
        )
        k = self.alphas[i] * scale
        nc.scalar.mul(s[:, lo:W], s[:, lo:W], k)
        return s

    def ema(self, xt, i, scale=1.0, lo=248):
        return self.ema_stage2(self.ema_stage1(xt, i, lo), scale)

    def ema_stage2_raw(self, st):
        """chained scan2 only; caller folds the alpha into a later scale."""
        nc = self.nc
        s, pcar, xt, i, lo = st
        oc = self.oCONSTS + i
        cbc2 = self.CONSTS_T[:, oc : oc + 1].broadcast_to([P, W - lo])
        nc.vector.tensor_tensor_scan(
            out=s[:, lo:W], data0=cbc2,
            data1=xt[:, lo:W], initial=pcar[:, 0:1], op0=Alu.mult, op1=Alu.add,
        )
        return s

    def ema_raw_pair(self, x1, i1, x2, i2, lo=248):
        f1 = self.ema_stage1(x1, i1, lo)
        f2 = self.ema_stage1(x2, i2, lo)
        return self.ema_stage2_raw(f1), self.ema_stage2_raw(f2)

    def scaled(self, s, k, lo=248):
        """materialize k*s into a fresh tile on ACT (off the scan chain)."""
        y = self.big()
        self.nc.scalar.mul(y[:, lo:W], s[:, lo:W], k)
        return y

    def ema_pair(self, x1, i1, x2, i2, scale1=1.0, scale2=1.0, lo=248):
        f1 = self.ema_stage1(x1, i1, lo)
        f2 = self.ema_stage1(x2, i2, lo)
        return self.ema_stage2(f1, scale1), self.ema_stage2(f2, scale2)

    # ---- sliding-window max/min ----
    def winchain(self, xt, jmax, op, alloc=None):
        """doubling chain for sliding max/min: dict 2^j -> (tile, lo)."""
        nc = self.nc
        alloc = alloc or self.big
        chain = {1: (xt, 0)}
        cur, curlo = xt, 0
        for j in range(jmax):
            sh = 1 << j
            dst = alloc()
            nc.vector.tensor_tensor(
                out=dst[:, curlo + sh : W], in0=cur[:, curlo + sh : W],
                in1=cur[:, curlo : W - sh], op=op,
            )
            cur, curlo = dst, curlo + sh
            chain[sh * 2] = (dst, curlo)
        return chain

    def wincombine(self, chain, n, op, alloc=None):
        """window-n result from a doubling chain."""
        nc = self.nc
        alloc = alloc or self.big
        J = 1 << int(math.floor(math.log2(n)))
        r = n - J
        cur, curlo = chain[J]
        out = alloc()
        if r > 0:
            nc.vector.tensor_tensor(
                out=out[:, curlo + r : W], in0=cur[:, curlo + r : W],
                in1=cur[:, curlo : W - r], op=op,
            )
        else:
            nc.vector.tensor_copy(out[:, curlo:W], cur[:, curlo:W])
        return out

    def winsum18(self, xt, need=C0, alloc=None):
        """rolling 18-window sum via doubling, trimmed to cols [need, W).
        W2 needed from need-32 (chain1 read) and each level from its reader."""
        nc = self.nc
        tiles = []
        # start cols per level, derived from the final read pattern
        alloc = alloc or self.big
        s2, s4, s8, s16 = need - 32, need - 24, need - 16, need - 8
        cur = xt
        chain1 = None
        for sh, st in ((1, s2), (2, s4), (4, s8), (8, s16)):
            dst = alloc()
            tiles.append(dst)
            nc.vector.tensor_add(
                dst[:, st:W], cur[:, st - sh : W - sh], cur[:, st:W]
            )
            cur = dst
            if sh == 1:
                chain1 = dst
        out = alloc()
        nc.vector.tensor_add(
            out[:, need:W], cur[:, need:W], chain1[:, need - 16 : W - 16]
        )
        for t in tiles:
            (self.rel if alloc == self.big else self.rel16)(t)
        return out, need

    # ---- full pipeline ----
    def build(self):
        nc = self.nc
        with ctile.TileContext(nc) as tc:
            with tc.tile_pool(name="big", bufs=1) as pool, tc.tile_pool(
                name="small", bufs=1
            ) as spool, tc.tile_pool(name="psc", bufs=4, space="PSUM") as pscol, \
                 tc.tile_pool(name="psr", bufs=2, space="PSUM") as psrow:
                self.pool = pool
                self.spool = spool
                self.pscol = pscol
                self.psrow = psrow
                self.emit()
        nc.finalize()
        return nc

    def mm(self, out, lhsT, rhs):
        self.nc.tensor.matmul(out, lhsT, rhs, start=True, stop=True)

    def emit(self):
        nc = self.nc
        # all big constants (GW weights, Sh1/Ident/M2, scan multipliers,
        # TG/TL1 iotas, m50) are host-precomputed and DMA'd in as one blob,
        # so nothing on the setup path gates the first EMA scans.
        wA = self.oGWlate
        wB = self.CBW - wA
        cbtA = self.spool.tile([P, wA], F32, tag="c_blobA")
        cbtB = self.spool.tile([P, wB], F32, tag="c_blobB")
        base = self.CB[0 : P * self.CBW].rearrange("(p w) -> p w", p=P, w=self.CBW)
        self.GW = {
            i: (cbtA[:, o : o + self.VD[i]] if i < 6 else
                cbtB[:, o - wA : o - wA + self.VD[i]])
            for i, o in self.gw_off.items()
        }
        self.CONSTS_T = cbtA
        self.Sh1 = cbtA[:, self.oSh1 : self.oSh1 + P]
        self.Ident = cbtA[:, self.oIdent : self.oIdent + P]
        self.M2 = {i: cbtA[:, o : o + P] for i, o in self.oM2.items()}
        self.m50 = cbtA[:, self.oM50 : self.oM50 + 1]
        self.TG = cbtB[:, self.oTG - wA : self.oTG - wA + CH]
        self.TL1 = cbtB[:, self.oTL1 - wA : self.oTL1 - wA + CH]
        self.ones11 = self.spool.tile([1, 1], F32, tag="c_ones11")
        nc.gpsimd.memset(self.ones11[:, :], 1.0)

        Ct = self.load_series(self.DC, eng="sync")
        Ht = self.load_series(self.DH, eng="scalar")
        Lt = self.load_series(self.DL, eng="gpsimd")
        nc.scalar.dma_start(out=cbtA[:, 0:wA], in_=base[:, 0:wA])
        nc.gpsimd.dma_start(out=cbtB[:, 0 : self.oTG - wA], in_=base[:, wA : self.oTG])
        nc.gpsimd.dma_start(out=cbtB[:, self.oTG - wA : wB], in_=base[:, self.oTG : self.CBW])

        # fp16 per-partition-anchor conversions first: they only need the
        # loads, and the H/L chains give DVE work while the GW blob lands.
        nb = self.small()
        nc.scalar.mul(nb[:, 0:1], Ct[:, 0:1], -1.0)
        h16 = self.big16()
        nc.scalar.activation(h16[:, 0:W], Ht[:, 0:W], Act.Identity, bias=nb[:, 0:1])
        l16 = self.big16()
        nc.scalar.activation(l16[:, 0:W], Lt[:, 0:W], Act.Identity, bias=nb[:, 0:1])
        c16 = self.big16()
        nc.scalar.activation(c16[:, 0:W], Ct[:, 0:W], Act.Identity, bias=nb[:, 0:1])
        self.rel(Ht, Lt)
        hchain = self.winchain(h16, 7, Alu.max, alloc=self.big16)
        lchain = self.winchain(l16, 7, Alu.min, alloc=self.big16)

        # --- rolling std of C (window 18), fp16 with per-partition anchor ---
        # dev = C - C[partition col 0]; var is shift-invariant, and the
        # TEU3/TED rows tolerate ~2 abs error, so fp16 sums are safe here.
        nb18 = self.small()
        nc.scalar.mul(nb18[:, 0:1], Ct[:, 0:1], -1.0 / 18.0)
        nbr18 = self.small()
        nc.scalar.mul(nbr18[:, 0:1], Ct[:, 0:1], -1.0 / math.sqrt(18.0))
        dev_s = self.big16()
        nc.scalar.activation(
            dev_s[:, 0:W], Ct[:, 0:W], Act.Identity, bias=nb18[:, 0:1],
            scale=1.0 / 18.0,
        )
        dev2 = self.big16()
        nc.scalar.activation(
            dev2[:, 0:W], Ct[:, 0:W], Act.Square, bias=nbr18[:, 0:1],
            scale=1.0 / math.sqrt(18.0),
        )
        m18, lo1 = self.winsum18(dev_s, alloc=self.big16)
        q18, lo2 = self.winsum18(dev2, alloc=self.big16)
        self.rel16(dev_s, dev2)
        mm2 = self.big16()
        nc.scalar.square(mm2[:, lo1:W], m18[:, lo1:W])
        var = self.big16()
        nc.vector.tensor_sub(var[:, lo1:W], q18[:, lo1:W], mm2[:, lo1:W])
        nc.vector.tensor_scalar_max(var[:, lo1:W], var[:, lo1:W], 0.0)
        DIS = self.big()
        nc.scalar.activation(DIS[:, lo1:W], var[:, lo1:W], Act.Sqrt)
        self.rel16(m18, q18, mm2, var)
        self.rels(nb18, nbr18)

        # --- TEMA3 + TEMAP2 chains (raw scans; alphas folded at the end) ---
        aa = self.alphas
        s1, t1 = self.ema_raw_pair(Ct, 0, Ct, 3, lo=240)
        s2, t2 = self.ema_raw_pair(s1, 1, t1, 4, lo=240)
        s3, t3 = self.ema_raw_pair(s2, 2, t2, 5, lo=240)

        EMA1 = self.scaled(s1, aa[0], lo=240)
        EMA2 = self.scaled(s2, aa[0] * aa[1], lo=240)
        EMA3 = self.scaled(s3, aa[0] * aa[1] * aa[2], lo=240)
        self.rel(s1, s2, s3)
        E21 = self.scaled(t1, aa[3], lo=240)
        E221 = self.scaled(t2, aa[3] * aa[4], lo=240)
        E231 = self.scaled(t3, aa[3] * aa[4] * aa[5], lo=240)
        self.rel(t1, t2, t3)
        TEMA3 = self.big()
        d = self.big()
        nc.vector.tensor_sub(d[:, 240:W], EMA1[:, 240:W], EMA2[:, 240:W])
        nc.vector.scalar_tensor_tensor(
            out=TEMA3[:, 240:W], in0=d[:, 240:W], scalar=3.0, in1=EMA3[:, 240:W],
            op0=Alu.mult, op1=Alu.add,
        )
        self.rel(EMA1, EMA2, EMA3, d)
        self.store_row(4, TEMA3)
        TEMAP2 = self.big()
        d = self.big()
        nc.vector.tensor_sub(d[:, 240:W], E21[:, 240:W], E221[:, 240:W])
        nc.vector.scalar_tensor_tensor(
            out=TEMAP2[:, 240:W], in0=d[:, 240:W], scalar=3.0, in1=E231[:, 240:W],
            op0=Alu.mult, op1=Alu.add,
        )
        self.rel(E21, E221, E231, d)

        # --- KDJ blocks (fp16, per-partition anchored) ---

        def rsvf(nw):
            """rsv in [0,100] via (C-ll)/(hh-ll)*100, fp16 anchored front end
            (the *100 is folded into the K-EMA scale)."""
            hh = self.wincombine(hchain, nw, Alu.max, alloc=self.big16)
            ll = self.wincombine(lchain, nw, Alu.min, alloc=self.big16)
            hl = self.big16()
            nc.vector.tensor_sub(hl[:, 248:W], hh[:, 248:W], ll[:, 248:W])
            hlf = self.big()
            nc.scalar.copy(hlf[:, 248:W], hl[:, 248:W])
            rcp = self.big()
            nc.vector.reciprocal_approx_fast(out=rcp[:, 248:W], in_=hlf[:, 248:W])
            rcp16 = self.big16()
            nc.scalar.copy(rcp16[:, 248:W], rcp[:, 248:W])
            num = self.big16()
            nc.vector.tensor_sub(num[:, 248:W], c16[:, 248:W], ll[:, 248:W])
            r0 = self.big16()
            nc.vector.tensor_mul(r0[:, 248:W], num[:, 248:W], rcp16[:, 248:W])
            self.rel(hlf, rcp)
            self.rel16(hh, ll, num, hl, rcp16)
            return r0

        def jof(Kv, Dv, rows):
            """J = 3K - 2D = 2*(K-D) + K; store K/D/J rows."""
            K16 = self.big16()
            nc.scalar.copy(K16[:, 248:W], Kv[:, 248:W])
            D16 = self.big16()
            nc.scalar.copy(D16[:, 248:W], Dv[:, 248:W])
            dkd = self.big16()
            nc.vector.tensor_sub(dkd[:, 248:W], K16[:, 248:W], D16[:, 248:W])
            self.rel16(K16, D16)
            Jv = self.big()
            nc.vector.scalar_tensor_tensor(
                out=Jv[:, 248:W], in0=dkd[:, 248:W], scalar=2.0, in1=Kv[:, 248:W],
                op0=Alu.mult, op1=Alu.add,
            )
            self.rel16(dkd)
            for ridx, tt_ in zip(rows, (Kv, Dv, Jv)):
                if ridx is not None:
                    self.store_row(ridx, tt_)
            return Jv

        a = self.alphas
        rsv1 = rsvf(204)
        rsv2 = rsvf(18)
        sK1, sK2 = self.ema_raw_pair(rsv1, 6, rsv2, 8)
        K1 = self.scaled(sK1, 100.0 * a[6])
        K2 = self.scaled(sK2, 100.0 * a[8])
        self.rel16(rsv1, rsv2)
        rsv3 = rsvf(9)
        rsvn = rsvf(36)
        sK3, sKN3 = self.ema_raw_pair(rsv3, 10, rsvn, 12)
        K3 = self.scaled(sK3, 100.0 * a[10])
        KN3 = self.scaled(sKN3, 100.0 * a[12])
        self.rel16(rsv3, rsvn)
        f1 = self.ema_stage1(sK1, 7)
        f2 = self.ema_stage1(sK2, 9)
        f3 = self.ema_stage1(sK3, 11)
        f4 = self.ema_stage1(sKN3, 13)
        D1 = self.ema_stage2(f1, 100.0 * a[6])
        D2 = self.ema_stage2(f2, 100.0 * a[8])
        D3 = self.ema_stage2(f3, 100.0 * a[10])
        DN3m2 = self.ema_stage2(f4, -2.0 * 100.0 * a[12])
        self.rel(sK1, sK2, sK3, sKN3)
        J1 = jof(K1, D1, (9, 10, 11))
        J2 = jof(K2, D2, (12, 13, 14))
        self.rel(K1, D1, K2, D2)
        J3 = jof(K3, D3, (15, 16, 17))
        JN3 = self.big()
        nc.vector.scalar_tensor_tensor(
            out=JN3[:, C0:W], in0=KN3[:, C0:W], scalar=3.0, in1=DN3m2[:, C0:W],
            op0=Alu.mult, op1=Alu.add,
        )
        self.store_row(18, JN3)
        self.rel(K3, D3, KN3, DN3m2, JN3)
        for ch_ in (hchain, lchain):
            for kk, (tt_, _) in ch_.items():
                if kk > 1:
                    self.rel16(tt_)
        self.rel16(h16, l16, c16)

        TEU3 = self.big()
        nc.vector.tensor_add(TEU3[:, C0:W], TEMA3[:, C0:W], DIS[:, C0:W])
        TED = self.big()
        nc.vector.tensor_sub(TED[:, C0:W], TEMA3[:, C0:W], DIS[:, C0:W])
        self.store_row(3, TEU3)
        self.store_row(5, TED)
        self.rel(DIS, TEU3, TED)

        # --- T ratios ---
        def recip_abs(xt, lo):
            ab = self.big()
            nc.scalar.activation(ab[:, lo:W], xt[:, lo:W], Act.Abs)
            rr = self.big()
            nc.vector.reciprocal_approx_fast(out=rr[:, lo:W], in_=ab[:, lo:W])
            self.rel(ab)
            return rr

        def tdiff(xt, lag, row_idx, rr16, lo=248):
            # rr16 = fp16(1/|xt|) (unshifted); read it lag-shifted
            dt_ = self.big16()
            nc.vector.tensor_sub(
                dt_[:, lo:W], xt[:, lo:W], xt[:, lo - lag : W - lag]
            )
            ts_ = self.big16()
            nc.vector.tensor_mul(ts_[:, lo:W], dt_[:, lo:W], rr16[:, lo - lag : W - lag])
            self.rel16(dt_)
            if row_idx is not None:
                tsf = self.big()
                nc.scalar.copy(tsf[:, C0:W], ts_[:, C0:W])
                self.store_row(row_idx, tsf)
                self.rel(tsf)
            return ts_

        rrA = recip_abs(TEMA3, 242)
        rrP = recip_abs(TEMAP2, 242)
        rrA16 = self.big16()
        nc.scalar.copy(rrA16[:, 242:W], rrA[:, 242:W])
        rrP16 = self.big16()
        nc.scalar.copy(rrP16[:, 242:W], rrP[:, 242:W])
        self.rel(rrA, rrP)
        T3s = tdiff(TEMA3, 6, 8, rrA16)
        T1s = tdiff(TEMA3, 1, 6, rrA16)
        T2s = tdiff(TEMAP2, 6, 7, rrP16)
        self.rel(TEMAP2, TEMA3)
        self.rel16(rrA16, rrP16)

        # --- JX family (J/T in fp16; JXb kept f32 for cross stability) ---
        J1_16 = self.big16()
        nc.scalar.copy(J1_16[:, 248:W], J1[:, 248:W])
        J2_16 = self.big16()
        nc.scalar.copy(J2_16[:, 248:W], J2[:, 248:W])
        J3_16 = self.big16()
        nc.scalar.copy(J3_16[:, 248:W], J3[:, 248:W])
        JXb = self.big()
        u = self.big16()
        nc.vector.tensor_mul(u[:, 248:W], J3_16[:, 248:W], T1s[:, 248:W])
        v = self.big()
        nc.vector.tensor_add(v[:, 248:W], J1[:, 248:W], J2[:, 248:W])
        nc.vector.tensor_add(JXb[:, 248:W], u[:, 248:W], v[:, 248:W])
        self.rel(v, J3, J1, J2)
        self.rel16(u, J3_16, T1s)
        F1 = self.big16()
        nc.vector.tensor_mul(F1[:, 248:W], J2_16[:, 248:W], T3s[:, 248:W])
        self.rel16(J2_16, T3s)
        F2 = self.big16()
        nc.vector.tensor_mul(F2[:, 248:W], J1_16[:, 248:W], T2s[:, 248:W])
        self.rel16(J1_16, T2s)
        self.store_row(19, JXb)
        F1f = self.big()
        nc.scalar.copy(F1f[:, C0:W], F1[:, C0:W])
        self.store_row(20, F1f)
        F2f = self.big()
        nc.scalar.copy(F2f[:, C0:W], F2[:, C0:W])
        self.store_row(21, F2f)
        self.rel(F1f, F2f)

        g1 = self.ema_stage1(JXb, 14)
        g2 = self.ema_stage1(F1, 15)
        g3 = self.ema_stage1(F2, 16)
        EMA_JX = self.ema_stage2(g1)
        EMA_F1 = self.ema_stage2(g2)
        EMA_F2 = self.ema_stage2(g3)
        self.store_row(22, EMA_JX)
        self.store_row(23, EMA_F1)
        self.store_row(24, EMA_F2)

        def jx_combine(base, f1, f2, row_idx, lo=254, keep_z=False):
            w_ = self.big()
            nc.vector.tensor_add(w_[:, lo:W], f1[:, lo:W], f2[:, lo:W])
            z = self.big()
            nc.vector.scalar_tensor_tensor(
                out=z[:, lo:W], in0=w_[:, lo:W], scalar=6.0, in1=base[:, lo:W],
                op0=Alu.mult, op1=Alu.add,
            )
            out = self.big()
            nc.scalar.activation(out[:, lo:W], z[:, lo:W], Act.Identity, bias=self.m50)
            self.rel(w_)
            if not keep_z:
                self.rel(z)
                z = None
            self.store_row(row_idx, out)
            return out, z

        # z_JX = JXb + 6*(F1+F2); EMAJX8 = a17*scan(z_JX) - 50 when the three
        # EMA8 alphas coincide (they do for the reference w_alphas); otherwise
        # fall back to three separate EMAs.
        a17, a18, a19 = self.alphas[17], self.alphas[18], self.alphas[19]
        fuse8 = abs(a17 - a18) < 1e-9 and abs(a17 - a19) < 1e-9
        JX, zJX = jx_combine(JXb, F1, F2, 27, lo=248, keep_z=True)
        if fuse8:
            st8 = self.ema_stage1(zJX, 17)
            oc8 = self.oCONSTS + 17
            cbc8 = self.CONSTS_T[:, oc8 : oc8 + 1].broadcast_to([P, W - 248])
            s8, pcar8 = st8[0], st8[1]
            nc.vector.tensor_tensor_scan(
                out=s8[:, 248:W], data0=cbc8, data1=zJX[:, 248:W],
                initial=pcar8[:, 0:1], op0=Alu.mult, op1=Alu.add,
            )
            EMAJX8 = self.big()
            nc.scalar.activation(
                EMAJX8[:, 248:W], s8[:, 248:W], Act.Identity,
                bias=self.m50, scale=a17,
            )
            self.rel(s8)
            self.store_row(29, EMAJX8)
        else:
            EMA8_JX = self.ema(JXb, 17)
            EMA8_F1 = self.ema(F1, 18)
            EMA8_F2 = self.ema(F2, 19)
            EMAJX8, _ = jx_combine(EMA8_JX, EMA8_F1, EMA8_F2, 29, lo=254)
            self.rel(EMA8_JX, EMA8_F1, EMA8_F2)
        self.rel(zJX)
        EMAJX, _ = jx_combine(EMA_JX, EMA_F1, EMA_F2, 28, lo=254)
        self.rel(JXb, EMA_JX, EMA_F1, EMA_F2)
        self.rel16(F1, F2)
        self.rel(EMAJX8)

        # --- crosses + segmented MA scans ---
        def macond(updown):
            """m_ = 1 - cond where cond = cross event mask.
            cross_up[t] = (JX>EMAJX)[t] AND NOT (JX>EMAJX)[t-1]  (is_le == !is_gt)
            cross_dn[t] = (JX<EMAJX)[t] AND NOT (JX<EMAJX)[t-1]"""
            g = self.big16()
            op = Alu.is_gt if updown == "up" else Alu.is_lt
            nc.vector.tensor_tensor(
                out=g[:, 254:W], in0=JX[:, 254:W], in1=EMAJX[:, 254:W], op=op,
            )
            w1 = self.big16()
            nc.vector.tensor_mul(w1[:, 255:W], g[:, 255:W], g[:, 254 : W - 1])
            m_ = self.big16()
            nc.vector.scalar_tensor_tensor(
                out=m_[:, 255:W], in0=w1[:, 255:W], scalar=1.0, in1=g[:, 255:W],
                op0=Alu.add, op1=Alu.subtract,
            )
            self.rel16(g, w1)
            return m_

        def ma_phase1(updown):
            m_ = macond(updown)
            dmask = self.big()
            nc.vector.tensor_mul(dmask[:, C0:W], Ct[:, C0:W], m_[:, C0:W])
            cnt_s = self.big()
            nc.vector.tensor_tensor_scan(
                out=cnt_s[:, C0:W], data0=m_[:, C0:W], data1=m_[:, C0:W],
                initial=0.0, op0=Alu.mult, op1=Alu.add,
            )
            acol = self.small()
            nc.vector.tensor_single_scalar(
                out=acol[:, 0:1], in_=cnt_s[:, W - 1 : W], scalar=float(CH),
                op=Alu.is_ge,
            )
            par = self.psrow.tile([1, P], F32, tag="psrow")
            self.mm(par[0:1, 0:P], acol[:, 0:1], self.Ident)
            arow = self.row()
            nc.vector.tensor_copy(arow[0:1, 0:P], par[0:1, 0:P])
            self.rels(acol)
            Ssum = self.big()
            nc.vector.tensor_tensor_scan(
                out=Ssum[:, C0:W], data0=m_[:, C0:W], data1=dmask[:, C0:W],
                initial=0.0, op0=Alu.mult, op1=Alu.add,
            )

            def carry_of(scan1_tile):
                rowt = self.row129()
                nc.gpsimd.memset(rowt[0:1, 0:1], 0.0)
                pbr = self.psrow.tile([1, P], F32, tag="psrow")
                self.mm(pbr[0:1, 0:P], scan1_tile[:, W - 1 : W], self.Ident)
                nc.vector.tensor_tensor_scan(
                    out=rowt[0:1, 1 : P + 1], data0=arow[0:1, 0:P],
                    data1=pbr[0:1, 0:P], initial=0.0, op0=Alu.mult, op1=Alu.add,
                )
                pcc = self.pscol.tile([P, 1], F32, tag="pscol")
                self.mm(pcc[:, 0:1], rowt[0:1, 0:P], self.ones11[0:1, 0:1])
                car = self.small()
                nc.vector.tensor_copy(car[:, 0:1], pcc[:, 0:1])
                self.relr129(rowt)
                return car

            carc = carry_of(cnt_s)
            cars = carry_of(Ssum)
            noev = self.big()
            nc.vector.tensor_tensor(
                out=noev[:, C0:W], in0=cnt_s[:, C0:W], in1=self.TL1,
                op=Alu.is_equal,
            )
            self.relr(arow)
            self.rel16(m_)
            self.rel(dmask)
            return dict(cnt_s=cnt_s, Ssum=Ssum, carc=carc, cars=cars, noev=noev)

        def ma_phase2(st, row_idx, diag_idx):
            cnt_s, Ssum, noev = st["cnt_s"], st["Ssum"], st["noev"]
            nc.vector.scalar_tensor_tensor(
                out=cnt_s[:, C0:W], in0=noev[:, C0:W], scalar=st["carc"][:, 0:1],
                in1=cnt_s[:, C0:W], op0=Alu.mult, op1=Alu.add,
            )
            nc.vector.scalar_tensor_tensor(
                out=Ssum[:, C0:W], in0=noev[:, C0:W], scalar=st["cars"][:, 0:1],
                in1=Ssum[:, C0:W], op0=Alu.mult, op1=Alu.add,
            )
            self.rels(st["carc"], st["cars"])
            seen = self.big()
            dcol = self.small()
            nc.vector.scalar_tensor_tensor(
                out=seen[:, C0:W], in0=cnt_s[:, C0:W], scalar=1.0,
                in1=self.TG, op0=Alu.mult, op1=Alu.is_le,
                accum_out=dcol[:, 0:1],
            )
            self.rel(noev)
            # diag early so its DMA/reduce overlaps the ma arithmetic
            drow = self.row()
            nc.sync.dma_start(out=drow[0:1, 0 : P - 1], in_=dcol[1:P, 0:1])
            done = self.spool.tile([1, 1], F32, tag=f"diag{diag_idx}", name=f"diag{diag_idx}")
            nc.vector.tensor_reduce(
                out=done[0:1, 0:1], in_=drow[0:1, 0 : P - 1],
                axis=mybir.AxisListType.X, op=Alu.min,
            )
            self.relr(drow)
            nc.sync.dma_start(
                out=self.DIAG[diag_idx : diag_idx + 1].rearrange(
                    "(a b) -> a b", a=1, b=1
                ),
                in_=done[0:1, 0:1],
            )
            self.rels(dcol)
            rc = self.big()
            nc.vector.tensor_scalar_max(rc[:, C0:W], cnt_s[:, C0:W], 1.0)
            rcp = self.big()
            nc.vector.reciprocal_approx_fast(out=rcp[:, C0:W], in_=rc[:, C0:W])
            ma0 = self.big()
            nc.vector.tensor_mul(ma0[:, C0:W], Ssum[:, C0:W], rcp[:, C0:W])
            ma = self.big()
            nc.vector.tensor_mul(ma[:, C0:W], ma0[:, C0:W], seen[:, C0:W])
            self.rel(rc, rcp, ma0, cnt_s, Ssum)
            self.store_row(row_idx, ma)
            self.rel(seen, ma)

        stA = ma_phase1("dn")
        stB = ma_phase1("up")
        ma_phase2(stA, 25, 1)
        ma_phase2(stB, 26, 0)
        self.rel(Ct, JX, EMAJX)


_CACHE = {}


def _build(alphas, anchor):
    key = (tuple(round(float(a), 12) for a in alphas), round(float(anchor), 6))
    if key not in _CACHE:
        kb = KB(alphas, anchor)
        _CACHE[key] = (kb.build(), kb.build_const_blob())
    return _CACHE[key]


def _shard(x):
    """per-core input arrays [DLEN], clamp-padded on the global left."""
    outs = []
    for mcore in range(NCORES):
        lo = (mcore + 1) * S - DLEN
        if lo < 0:
            d = np.concatenate(
                [np.full(-lo, x[0], np.float32), x[0 : (mcore + 1) * S]]
            )
        else:
            d = x[lo : (mcore + 1) * S]
        outs.append(np.ascontiguousarray(d, np.float32))
    return outs


def _host_ma(C, JX, EJ):
    """exact host fallback for ma rows (numpy, global)."""
    f32 = np.float32
    T_ = len(C)
    lag = lambda x: np.concatenate([x[:1], x[:-1]])
    JXp, EJp = lag(JX), lag(EJ)
    res = {}
    cs = np.concatenate([[0.0], np.cumsum(C.astype(np.float64))])
    t_idx = np.arange(T_)
    for key, cond in (
        ("dn", (JX < EJ) & (JXp >= EJp)),
        ("up", (JX > EJ) & (JXp <= EJp)),
    ):
        last = np.maximum.accumulate(np.where(cond, t_idx, -1))
        csl = cs[np.maximum(last, 0) + 1]
        s = cs[t_idx + 1] - csl
        n = t_idx - last
        res[key] = np.where(
            (last >= 0) & (n > 0), s / np.maximum(n, 1), 0.0
        ).astype(f32)
    return res["dn"], res["up"]


def run_cores(inputs, trace=False):
    """compile (cached) + run on 8 cores; returns BassKernelResults."""
    C = np.ascontiguousarray(inputs["C"], np.float32)
    H = np.ascontiguousarray(inputs["H"], np.float32)
    L = np.ascontiguousarray(inputs["L"], np.float32)
    w = np.asarray(inputs["w_alphas"], np.float32)
    alphas = [float(1.0 / (1.0 + math.exp(-float(x)))) for x in w]
    nc, cb = _build(alphas, float(C[0]))
    dc, dh, dl = _shard(C), _shard(H), _shard(L)
    in_maps = [
        {"DC": dc[m], "DH": dh[m], "DL": dl[m], "CB": cb} for m in range(NCORES)
    ]
    res = run_bass_kernel_spmd(
        nc, in_maps, core_ids=list(range(NCORES)), trace=trace
    )
    return res


def kernel(C, H, L, w_alphas):
    inputs = {"C": C, "H": H, "L": L, "w_alphas": w_alphas}
    res = run_cores(inputs)
    outs = [res.results[m]["OUT"].reshape(NROWS, EXT)[:, HALO:] for m in range(NCORES)]
    full = np.concatenate(outs, axis=1)
    full[0] = np.asarray(C, np.float32)
    full[1] = np.asarray(H, np.float32)
    full[2] = np.asarray(L, np.float32)

    # host patch: reference's partial-window std for the first 17 bars
    Cg = np.asarray(C, np.float64)[:17]
    for t in range(17):
        wdw = Cg[: t + 1]
        dis = math.sqrt(max(np.mean(wdw * wdw) - np.mean(wdw) ** 2, 0.0))
        full[3, t] = np.float32(full[4, t] + dis)
        full[5, t] = np.float32(full[4, t] - dis)

    # diag check: cross gap exceeded the halo on some core -> exact host fix
    need_fix = False
    for mcore in range(1, NCORES):
        dg = res.results[mcore]["DIAG"]
        if dg.min() < CH - 0.5:
            need_fix = True
    if need_fix:
        ma_dn, ma_up = _host_ma(
            np.asarray(C, np.float32), full[27], full[28]
        )
        full[25] = ma_dn
        full[26] = ma_up
    return full.astype(np.float32)
